# revision 12
# baseline (speedup 1.0000x reference)
# Chunkwise delta-rule attention kernel for Trainium2 (Bass/Tile), 8 NeuronCores.
#
# Sharding: 8 cores = 4 batches x 2 head-groups (8 heads each). Each core:
#   - projects x[b] (4096x1024) against its 512-row weight slices (q,k,v,beta,alpha)
#   - runs the chunked intra-attention + sequential inter-chunk state scan
#   - produces y[b,:,hg*512:(hg+1)*512], state[b, hg*8:(hg+1)*8], alpha_val slice
#
# Layout strategy (contraction dim must be the SBUF partition dim for matmul):
#   - x and W are pre-transposed and cast to bf16 on the host -> xT [1024,4096],
#     wT [1024,512]. q/k are produced transposed ([head_dim, t]) for the attn
#     matmul; v/beta/alpha are produced natural ([t, dim]). k_nat is recovered
#     from kT with on-device bf16 DMA transposes.
#   - decay exp(m_i - m_j) is factored as curve_i * invcurve_j; curve_i is
#     folded into the final per-row y scale, invcurve_j into the attn matrix.
#   - cumsum over chunk positions is a matmul with a triangular constant.

import numpy as np

B, T, D = 4, 4096, 1024
HEAD_DIM, CHUNK = 64, 128
H = D // HEAD_DIM  # 16 heads total
HG = 2  # head groups (cores per batch)
OC = D // HG  # 512 per-core projected dim
N_CORES = 8
STRIPE = 512
N_STRIPES = T // STRIPE  # 8
CPS = STRIPE // CHUNK  # chunks per stripe = 4
NT = STRIPE // 128  # t-tiles per stripe = 4
KT = D // 128  # k(contraction) tiles = 8
OT = OC // 128  # o-tiles = 4 (one per head pair)
QSCALE = float(HEAD_DIM) ** -0.5  # 0.125

_PROGRAM_CACHE = {}
LAST_RESULTS = None  # BassKernelResults of the most recent run (for test harness)


def build_program():
    import concourse.mybir as mybir
    import concourse.tile as tile
    from concourse import bacc

    f32 = mybir.dt.float32
    bf16 = mybir.dt.bfloat16
    ALU = mybir.AluOpType
    ACT = mybir.ActivationFunctionType
    AX = mybir.AxisListType

    nc = bacc.Bacc()

    # ---- DRAM I/O -----------------------------------------------------------
    xT = nc.declare_dram_parameter("xT", [D, T], bf16, isOutput=False)
    wT = [
        nc.declare_dram_parameter(f"wT{i}", [D, OC], bf16, isOutput=False)
        for i in range(5)
    ]  # order: q, k, v, beta, alpha
    bq_pp = nc.declare_dram_parameter("bq_pp", [128, OT], f32, isOutput=False)
    bk_pp = nc.declare_dram_parameter("bk_pp", [128, OT], f32, isOutput=False)
    bv_bc = nc.declare_dram_parameter("bv_bc", [128, OC], f32, isOutput=False)
    bb_bc = nc.declare_dram_parameter("bb_bc", [128, OC], f32, isOutput=False)
    ba_bc = nc.declare_dram_parameter("ba_bc", [128, OC], f32, isOutput=False)
    mask_ut = nc.declare_dram_parameter("mask_ut", [128, 128], f32, isOutput=False)
    tri_sc = nc.declare_dram_parameter("tri_sc", [128, 128], f32, isOutput=False)
    ones_sc = nc.declare_dram_parameter("ones_sc", [128, 128], f32, isOutput=False)

    y_out = nc.declare_dram_parameter("y_out", [T, OC], f32, isOutput=True)
    av_out = nc.declare_dram_parameter("av_out", [T, OC], f32, isOutput=True)
    st_out = nc.declare_dram_parameter("st_out", [4 * 128, HEAD_DIM], f32, isOutput=True)

    with tile.TileContext(nc) as tc:
        import contextlib

        ctx = contextlib.ExitStack()
        with ctx:
            consts = ctx.enter_context(tc.tile_pool(name="consts", bufs=1))
            xpool = ctx.enter_context(tc.tile_pool(name="xpool", bufs=2))
            qkpool = ctx.enter_context(tc.tile_pool(name="qkpool", bufs=2))
            actpool = ctx.enter_context(tc.tile_pool(name="actpool", bufs=2))
            scratch = ctx.enter_context(tc.tile_pool(name="scratch", bufs=3))
            small = ctx.enter_context(tc.tile_pool(name="small", bufs=3))
            outpool = ctx.enter_context(tc.tile_pool(name="outpool", bufs=2))
            ps_proj = ctx.enter_context(tc.tile_pool(name="ps_proj", bufs=2, space="PSUM"))
            ps_attn = ctx.enter_context(tc.tile_pool(name="ps_attn", bufs=3, space="PSUM"))
            ps_small = ctx.enter_context(tc.tile_pool(name="ps_small", bufs=3, space="PSUM"))

            # ---- constants / weights into SBUF ------------------------------
            # first stripe of xT precedes the weights in DMA priority order so
            # the PE can start projecting as early as possible
            xsb0 = []
            for it in range(KT):
                xt0 = xpool.tile([128, STRIPE], bf16, tag=f"x{it}")
                nc.sync.dma_start(out=xt0[:], in_=xT[it * 128:(it + 1) * 128, 0:STRIPE])
                xsb0.append(xt0)

            wsb = []  # wsb[w][it] : [128, OC] bf16
            for w in range(5):
                tiles = []
                for it in range(KT):
                    t = consts.tile([128, OC], bf16, tag=f"w{w}_{it}")
                    nc.sync.dma_start(out=t[:], in_=wT[w][it * 128:(it + 1) * 128, :])
                    tiles.append(t)
                wsb.append(tiles)

            sb_bq = consts.tile([128, OT], f32, tag="bq")
            nc.sync.dma_start(out=sb_bq[:], in_=bq_pp[:])
            sb_bk = consts.tile([128, OT], f32, tag="bk")
            nc.sync.dma_start(out=sb_bk[:], in_=bk_pp[:])
            sb_bv = consts.tile([128, OC], f32, tag="bv")
            nc.sync.dma_start(out=sb_bv[:], in_=bv_bc[:])
            sb_bb = consts.tile([128, OC], f32, tag="bb")
            nc.sync.dma_start(out=sb_bb[:], in_=bb_bc[:])
            sb_ba = consts.tile([128, OC], f32, tag="ba")
            nc.sync.dma_start(out=sb_ba[:], in_=ba_bc[:])
            sb_mask = consts.tile([128, 128], f32, tag="mask")
            nc.sync.dma_start(out=sb_mask[:], in_=mask_ut[:])
            sb_tri = consts.tile([128, 128], f32, tag="tri")
            nc.sync.dma_start(out=sb_tri[:], in_=tri_sc[:])
            sb_ones = consts.tile([128, 128], f32, tag="ones")
            nc.sync.dma_start(out=sb_ones[:], in_=ones_sc[:])

            # per-pair state: [128, 64] (heads 2g, 2g+1 stacked on partitions)
            st32 = []
            st16 = []
            for g in range(OT):
                s32 = consts.tile([128, HEAD_DIM], f32, tag=f"st32_{g}")
                s16 = consts.tile([128, HEAD_DIM], bf16, tag=f"st16_{g}")
                nc.vector.memset(s32[:], 0.0)
                nc.vector.memset(s16[:], 0.0)
                st32.append(s32)
                st16.append(s16)

            # ---- main loop over stripes ------------------------------------
            for s in range(N_STRIPES):
                t0 = s * STRIPE

                # load xT stripe (stripe 0 was preloaded before the weights)
                if s == 0:
                    xsb = xsb0
                else:
                    xsb = []
                    for it in range(KT):
                        xt = xpool.tile([128, STRIPE], bf16, tag=f"x{it}")
                        nc.sync.dma_start(out=xt[:], in_=xT[it * 128:(it + 1) * 128, t0:t0 + STRIPE])
                        xsb.append(xt)

                # transposed projections -> qT, kT  ([o-part, t] bf16)
                qT, kT_ = [], []
                for w in range(2):
                    bias = sb_bq if w == 0 else sb_bk
                    for ot in range(OT):
                        ps = ps_proj.tile([128, STRIPE], f32, tag="proj")
                        for it in range(KT):
                            nc.tensor.matmul(
                                ps[:],
                                lhsT=wsb[w][it][:, ot * 128:(ot + 1) * 128],
                                rhs=xsb[it][:],
                                start=(it == 0),
                                stop=(it == KT - 1),
                            )
                        dst = qkpool.tile([128, STRIPE], bf16, tag=f"qk{w}_{ot}")
                        nc.scalar.activation(
                            dst[:], ps[:], ACT.Identity,
                            bias=bias[:, ot:ot + 1], scale=QSCALE,
                        )
                        (qT if w == 0 else kT_).append(dst)

                # k natural layout via bf16 DMA transpose of kT
                knat = []
                for tt in range(NT):
                    kn = actpool.tile([128, OC], bf16, tag=f"knat{tt}")
                    for ot in range(OT):
                        nc.sync.dma_start(
                            out=kn[:, ot * 128:(ot + 1) * 128],
                            in_=kT_[ot][:, tt * 128:(tt + 1) * 128],
                            transpose=True,
                        )
                    knat.append(kn)

                # natural projections (v, beta, alpha) + per-chunk stats
                vts, vbs, vbms, lasums = [], [], [], []
                for tt in range(NT):
                    psn = {}
                    for w in (2, 3, 4):
                        ps = ps_proj.tile([128, OC], f32, tag="proj")
                        for it in range(KT):
                            nc.tensor.matmul(
                                ps[:],
                                lhsT=xsb[it][:, tt * 128:(tt + 1) * 128],
                                rhs=wsb[w][it][:],
                                start=(it == 0),
                                stop=(it == KT - 1),
                            )
                        psn[w] = ps

                    # v (keep f32 for precise vb/vbm products)
                    v_t = scratch.tile([128, OC], f32, tag="v")
                    nc.vector.tensor_tensor(v_t[:], psn[2][:], sb_bv[:], ALU.add)

                    # beta = softplus(pre + bias) = ln(1 + exp(pre + bias))
                    # (only Exp/Ln/Identity ACT funcs are used anywhere in the
                    # kernel: they share one activation table -> no reloads)
                    tb = scratch.tile([128, OC], f32, tag="tb")
                    nc.vector.tensor_tensor(tb[:], psn[3][:], sb_bb[:], ALU.add)
                    ebt = scratch.tile([128, OC], f32, tag="ebt")
                    nc.scalar.activation(ebt[:], tb[:], ACT.Exp)
                    beta_t = scratch.tile([128, OC], f32, tag="beta")
                    nc.scalar.activation(beta_t[:], ebt[:], ACT.Ln, bias=1.0)

                    # alpha: clip(pre + bias, +-10)
                    ta = scratch.tile([128, OC], f32, tag="ta")
                    nc.vector.tensor_tensor(ta[:], psn[4][:], sb_ba[:], ALU.add)
                    tcl = scratch.tile([128, OC], f32, tag="tcl")
                    nc.vector.tensor_scalar(tcl[:], ta[:], 10.0, -10.0, ALU.min, ALU.max)

                    # u = exp(-logits); alpha_val = 1/(1+u); -log_alpha = ln(1+u)
                    ua = scratch.tile([128, OC], f32, tag="ua")
                    nc.scalar.activation(ua[:], tcl[:], ACT.Exp, scale=-1.0)
                    up1 = scratch.tile([128, OC], f32, tag="up1")
                    nc.vector.tensor_scalar(up1[:], ua[:], 1.0, None, ALU.add)
                    av_t = outpool.tile([128, OC], f32, tag="av")
                    nc.vector.reciprocal(av_t[:], up1[:])
                    nc.sync.dma_start(
                        out=av_out[t0 + tt * 128: t0 + (tt + 1) * 128, :], in_=av_t[:]
                    )

                    # sum_d -log_alpha = sum_d ln(1+u)
                    sp_t = scratch.tile([128, OC], f32, tag="sp")
                    nc.scalar.activation(sp_t[:], ua[:], ACT.Ln, bias=1.0)
                    lasum = small.tile([128, H // HG], f32, tag=f"lasum{tt}")
                    nc.vector.tensor_reduce(
                        lasum[:],
                        sp_t[:].rearrange("p (h d) -> p h d", h=H // HG),
                        AX.X, ALU.add,
                    )

                    # bmean = sum_d beta / 64
                    bsum = small.tile([128, H // HG], f32, tag=f"bsum{tt}")
                    nc.vector.tensor_reduce(
                        bsum[:],
                        beta_t[:].rearrange("p (h d) -> p h d", h=H // HG),
                        AX.X, ALU.add,
                    )
                    bm = small.tile([128, H // HG], f32, tag=f"bm{tt}")
                    nc.vector.tensor_scalar(bm[:], bsum[:], 1.0 / HEAD_DIM, None, ALU.mult)

                    # vb = v*beta (bf16), vbm = v*bmean (bf16)
                    vb_t = actpool.tile([128, OC], bf16, tag=f"vb{tt}")
                    nc.vector.tensor_tensor(vb_t[:], v_t[:], beta_t[:], ALU.mult)
                    vbm_t = actpool.tile([128, OC], bf16, tag=f"vbm{tt}")
                    nc.vector.tensor_tensor(
                        vbm_t[:].rearrange("p (h d) -> p h d", h=H // HG),
                        v_t[:].rearrange("p (h d) -> p h d", h=H // HG),
                        bm[:, :, None].to_broadcast((128, H // HG, HEAD_DIM)),
                        ALU.mult,
                    )
                    vts.append(v_t)
                    vbs.append(vb_t)
                    vbms.append(vbm_t)
                    lasums.append(lasum)

                # ---- attention per chunk (chunk == t-tile) ------------------
                for cc in range(CPS):
                    tsl = cc * 128
                    lasum = lasums[cc]

                    # m[c,h] (cumulative mean log-alpha) and its all-rows bcast
                    ps_m = ps_small.tile([128, H // HG], f32, tag="sm")
                    nc.tensor.matmul(ps_m[:], lhsT=sb_tri[:], rhs=lasum[:])
                    ps_ms = ps_small.tile([128, H // HG], f32, tag="sm")
                    nc.tensor.matmul(ps_ms[:], lhsT=sb_ones[:], rhs=lasum[:])

                    curve = small.tile([128, H // HG], f32, tag="curve")
                    nc.scalar.activation(curve[:], ps_m[:], ACT.Exp)
                    invc = small.tile([128, H // HG], f32, tag="invc")
                    nc.scalar.activation(invc[:], ps_m[:], ACT.Exp, scale=-1.0)
                    asum = small.tile([128, H // HG], f32, tag="asum")
                    nc.scalar.activation(asum[:], ps_ms[:], ACT.Exp)

                    y_f = outpool.tile([128, OC], f32, tag="yf")

                    for g in range(OT):
                        qg, kg = qT[g], kT_[g]
                        # attn_T[j,i] for both heads of the pair
                        ps_a = [None, None]
                        for hh in range(2):
                            r0, r1 = hh * 64, (hh + 1) * 64
                            pa = ps_attn.tile([128, 128], f32, tag="attn")
                            nc.tensor.matmul(
                                pa[:],
                                lhsT=kg[r0:r1, tsl:tsl + 128],
                                rhs=qg[r0:r1, tsl:tsl + 128],
                            )
                            ps_a[hh] = pa

                        for hh in range(2):
                            h = 2 * g + hh
                            r0, r1 = hh * 64, (hh + 1) * 64
                            # scale cols... rows j by invcurve_j, mask j<=i, ->bf16
                            at0 = scratch.tile([128, 128], f32, tag="at0")
                            nc.scalar.activation(
                                at0[:], ps_a[hh][:], ACT.Identity,
                                scale=invc[:, h:h + 1],
                            )
                            atm = scratch.tile([128, 128], bf16, tag="atm")
                            nc.vector.tensor_tensor(atm[:], at0[:], sb_mask[:], ALU.mult)

                            # y_psum = attn_tilde^T . vb  +  qT^T . state
                            ps_y = ps_small.tile([128, HEAD_DIM], f32, tag="sm")
                            nc.tensor.matmul(
                                ps_y[:],
                                lhsT=atm[:],
                                rhs=vbs[cc][:, h * 64:(h + 1) * 64],
                                start=True, stop=False,
                            )
                            nc.tensor.matmul(
                                ps_y[:],
                                lhsT=qg[r0:r1, tsl:tsl + 128],
                                rhs=st16[g][r0:r1, :],
                                start=False, stop=True,
                            )
                            # y = curve_i * psum
                            nc.scalar.activation(
                                y_f[:, h * 64:(h + 1) * 64], ps_y[:], ACT.Identity,
                                scale=curve[:, h:h + 1],
                            )

                        # delta for the pair (col-tiled outer products)
                        ps_d = ps_small.tile([128, HEAD_DIM], f32, tag="sm")
                        for hh in range(2):
                            h = 2 * g + hh
                            nc.tensor.matmul(
                                ps_d[hh * 64:(hh + 1) * 64, :],
                                lhsT=knat[cc][:, h * 64:(h + 1) * 64],
                                rhs=vbms[cc][:, h * 64:(h + 1) * 64],
                            )

                        # state = state * asum + delta   (f32, then bf16 copy)
                        for hh in range(2):
                            h = 2 * g + hh
                            r0, r1 = hh * 64, (hh + 1) * 64
                            nc.vector.tensor_scalar(
                                st32[g][r0:r1, :], st32[g][r0:r1, :],
                                asum[r0:r1, h:h + 1], None, ALU.mult,
                            )
                            nc.vector.tensor_tensor(
                                st32[g][r0:r1, :], st32[g][r0:r1, :],
                                ps_d[r0:r1, :], ALU.add,
                            )
                        nc.vector.tensor_copy(st16[g][:], st32[g][:])

                    nc.sync.dma_start(
                        out=y_out[t0 + tsl: t0 + tsl + 128, :], in_=y_f[:]
                    )

            # final state -> DRAM
            for g in range(OT):
                nc.sync.dma_start(out=st_out[g * 128:(g + 1) * 128, :], in_=st32[g][:])

    nc.compile()
    return nc


def _get_program():
    if "nc" not in _PROGRAM_CACHE:
        _PROGRAM_CACHE["nc"] = build_program()
    return _PROGRAM_CACHE["nc"]


def make_core_inputs(x, Wq, bq, Wk, bk, Wv, bv, Wbeta, bbeta, Walpha, balpha):
    """Host-side shard prep: returns in_maps (list of 8 dicts)."""
    import ml_dtypes

    bf16 = ml_dtypes.bfloat16
    f32 = np.float32

    # constants shared by all cores
    idx = np.arange(128)
    mask_ut = (idx[:, None] <= idx[None, :]).astype(f32)  # [j, i] : j <= i
    # lasum holds -sum_d(log_alpha); fold the sign and the /64 into the
    # cumulative-sum matmul constants
    tri = np.triu(np.ones((128, 128), f32)) * (-1.0 / HEAD_DIM)  # c' <= c
    ones_sc = np.full((128, 128), -1.0 / HEAD_DIM, f32)

    Ws = [Wq, Wk, Wv, Wbeta, Walpha]
    in_maps = []
    for core in range(N_CORES):
        b, hg = divmod(core, HG)
        sl = slice(hg * OC, (hg + 1) * OC)
        m = {
            "xT": np.ascontiguousarray(np.asarray(x[b]).T).astype(bf16),
            "mask_ut": mask_ut,
            "tri_sc": tri,
            "ones_sc": ones_sc,
        }
        for i, W in enumerate(Ws):
            m[f"wT{i}"] = np.ascontiguousarray(np.asarray(W)[sl].T).astype(bf16)
        m["bq_pp"] = np.ascontiguousarray(
            (np.asarray(bq)[sl] * QSCALE).astype(f32).reshape(OT, 128).T
        )
        m["bk_pp"] = np.ascontiguousarray(
            (np.asarray(bk)[sl] * QSCALE).astype(f32).reshape(OT, 128).T
        )
        m["bv_bc"] = np.ascontiguousarray(
            np.broadcast_to(np.asarray(bv)[sl].astype(f32), (128, OC))
        )
        m["bb_bc"] = np.ascontiguousarray(
            np.broadcast_to(np.asarray(bbeta)[sl].astype(f32), (128, OC))
        )
        m["ba_bc"] = np.ascontiguousarray(
            np.broadcast_to(np.asarray(balpha)[sl].astype(f32), (128, OC))
        )
        in_maps.append(m)
    return in_maps


def assemble_outputs(results):
    """results: list of 8 dicts with y_out/av_out/st_out -> full (y, state, alpha_val)."""
    y = np.zeros((B, T, D), np.float32)
    state = np.zeros((B, H, HEAD_DIM, HEAD_DIM), np.float32)
    alpha_val = np.zeros((B, T, H, HEAD_DIM), np.float32)
    for core in range(N_CORES):
        b, hg = divmod(core, HG)
        r = results[core]
        y[b, :, hg * OC:(hg + 1) * OC] = r["y_out"]
        alpha_val[b, :, hg * 8:(hg + 1) * 8, :] = r["av_out"].reshape(T, 8, HEAD_DIM)
        state[b, hg * 8:(hg + 1) * 8] = r["st_out"].reshape(8, HEAD_DIM, HEAD_DIM)
    return y, state, alpha_val


def _ensure_ntff_hook():
    """This image lacks the antenv.axon_hooks glue module; synthesize it so
    run_bass_kernel_spmd(trace=True) can reach the ctypes NTFF hook."""
    import sys
    import types

    if "antenv.axon_hooks" in sys.modules:
        return
    try:
        import antenv

        mod = types.ModuleType("antenv.axon_hooks")
        _h = [None]
        mod.set_axon_ntff_profile_hook = lambda h: _h.__setitem__(0, h)
        mod.get_axon_ntff_profile_hook = lambda: _h[0]
        sys.modules["antenv.axon_hooks"] = mod
        antenv.axon_hooks = mod
        from trn_agent_boot.trn_boot import _ntff_profile_via_ctypes

        mod.set_axon_ntff_profile_hook(
            _ntff_profile_via_ctypes("/opt/axon/libaxon_pjrt.so")
        )
    except Exception:
        pass


def run(inputs, trace=False):
    global LAST_RESULTS
    import jax  # ensures the axon PJRT platform is initialized

    jax.devices()
    _ensure_ntff_hook()
    from concourse.bass_utils import run_bass_kernel_spmd

    nc = _get_program()
    in_maps = make_core_inputs(**inputs)
    res = run_bass_kernel_spmd(
        nc, in_maps, list(range(N_CORES)), trace=trace
    )
    LAST_RESULTS = res
    return res


def kernel(x, Wq, bq, Wk, bk, Wv, bv, Wbeta, bbeta, Walpha, balpha):
    res = run(dict(
        x=x, Wq=Wq, bq=bq, Wk=Wk, bk=bk, Wv=Wv, bv=bv, Wbeta=Wbeta,
        bbeta=bbeta, Walpha=Walpha, balpha=balpha,
    ))
    return assemble_outputs(res.results)


# revision 14
# speedup vs baseline: 1.3000x; 1.3000x over previous
# Chunkwise delta-rule attention kernel for Trainium2 (Bass/Tile), 8 NeuronCores.
#
# Sharding: 8 cores = 4 batches x 2 head-groups (8 heads each). Each core:
#   - projects x[b] (4096x1024) against its 512-row weight slices (q,k,v,beta,alpha)
#   - runs the chunked intra-attention + sequential inter-chunk state scan
#   - produces y[b,:,hg*512:(hg+1)*512], state[b, hg*8:(hg+1)*8], alpha_val slice
#
# Layout strategy (contraction dim must be the SBUF partition dim for matmul):
#   - x and W are pre-transposed and cast to bf16 on the host -> xT [1024,4096],
#     wT [1024,512]. q/k are produced transposed ([head_dim, t]) for the attn
#     matmul; v/beta/alpha are produced natural ([t, dim]). k_nat is recovered
#     from kT with on-device bf16 DMA transposes.
#   - decay exp(m_i - m_j) is factored as curve_i * invcurve_j; curve_i is
#     folded into the final per-row y scale, invcurve_j into the attn matrix.
#   - cumsum over chunk positions is a matmul with a triangular constant.

import numpy as np

B, T, D = 4, 4096, 1024
HEAD_DIM, CHUNK = 64, 128
H = D // HEAD_DIM  # 16 heads total
HG = 2  # head groups (cores per batch)
OC = D // HG  # 512 per-core projected dim
N_CORES = 8
STRIPE = 512
N_STRIPES = T // STRIPE  # 8
CPS = STRIPE // CHUNK  # chunks per stripe = 4
NT = STRIPE // 128  # t-tiles per stripe = 4
KT = D // 128  # k(contraction) tiles = 8
OT = OC // 128  # o-tiles = 4 (one per head pair)
QSCALE = float(HEAD_DIM) ** -0.5  # 0.125

_PROGRAM_CACHE = {}
LAST_RESULTS = None  # BassKernelResults of the most recent run (for test harness)


def build_program():
    import concourse.mybir as mybir
    import concourse.tile as tile
    from concourse import bacc

    f32 = mybir.dt.float32
    bf16 = mybir.dt.bfloat16
    ALU = mybir.AluOpType
    ACT = mybir.ActivationFunctionType
    AX = mybir.AxisListType

    # The act-table chooser greedily picks the first act_info.json set that
    # contains each function, which thrashes between the exp-only and ln-only
    # tables. Every ACT func this kernel uses (Exp, Ln, Identity, Copy) lives
    # in 'natural_log_exp_and_others'; blank out the other sets (keeping list
    # positions, since act_func_set_id is the index) so one table is loaded
    # once and never evicted.
    import concourse.bacc as bacc_mod

    if not hasattr(bacc_mod, "_orig_get_activation_tables"):
        bacc_mod._orig_get_activation_tables = bacc_mod.get_activation_tables

        def _only_ln_exp_tables(module_arch):
            tabs = bacc_mod._orig_get_activation_tables(module_arch)
            return {
                name: (s if name == "natural_log_exp_and_others" else set())
                for name, s in tabs.items()
            }

        bacc_mod.get_activation_tables = _only_ln_exp_tables

    nc = bacc.Bacc()

    # ---- DRAM I/O -----------------------------------------------------------
    xT = nc.declare_dram_parameter("xT", [D, T], bf16, isOutput=False)
    wT = [
        nc.declare_dram_parameter(f"wT{i}", [D, OC], bf16, isOutput=False)
        for i in range(5)
    ]  # order: q, k, v, beta, alpha
    bq_pp = nc.declare_dram_parameter("bq_pp", [128, OT], f32, isOutput=False)
    bk_pp = nc.declare_dram_parameter("bk_pp", [128, OT], f32, isOutput=False)
    bv_bc = nc.declare_dram_parameter("bv_bc", [128, OC], f32, isOutput=False)
    bb_bc = nc.declare_dram_parameter("bb_bc", [128, OC], f32, isOutput=False)
    ba_bc = nc.declare_dram_parameter("ba_bc", [128, OC], f32, isOutput=False)
    mask_ut = nc.declare_dram_parameter("mask_ut", [128, 128], f32, isOutput=False)
    tri_sc = nc.declare_dram_parameter("tri_sc", [128, 128], f32, isOutput=False)
    ones_sc = nc.declare_dram_parameter("ones_sc", [128, 128], f32, isOutput=False)

    y_out = nc.declare_dram_parameter("y_out", [T, OC], f32, isOutput=True)
    av_out = nc.declare_dram_parameter("av_out", [T, OC], f32, isOutput=True)
    st_out = nc.declare_dram_parameter("st_out", [4 * 128, HEAD_DIM], f32, isOutput=True)

    with tile.TileContext(nc) as tc:
        import contextlib

        ctx = contextlib.ExitStack()
        with ctx:
            consts = ctx.enter_context(tc.tile_pool(name="consts", bufs=1))
            xpool = ctx.enter_context(tc.tile_pool(name="xpool", bufs=2))
            qkpool = ctx.enter_context(tc.tile_pool(name="qkpool", bufs=2))
            actpool = ctx.enter_context(tc.tile_pool(name="actpool", bufs=2))
            scratch = ctx.enter_context(tc.tile_pool(name="scratch", bufs=3))
            small = ctx.enter_context(tc.tile_pool(name="small", bufs=3))
            outpool = ctx.enter_context(tc.tile_pool(name="outpool", bufs=2))
            ps_proj = ctx.enter_context(tc.tile_pool(name="ps_proj", bufs=2, space="PSUM"))
            ps_attn = ctx.enter_context(tc.tile_pool(name="ps_attn", bufs=3, space="PSUM"))
            ps_small = ctx.enter_context(tc.tile_pool(name="ps_small", bufs=3, space="PSUM"))

            # ---- constants / weights into SBUF ------------------------------
            # first stripe of xT precedes the weights in DMA priority order so
            # the PE can start projecting as early as possible
            xsb0 = []
            for it in range(KT):
                xt0 = xpool.tile([128, STRIPE], bf16, tag=f"x{it}")
                nc.sync.dma_start(out=xt0[:], in_=xT[it * 128:(it + 1) * 128, 0:STRIPE])
                xsb0.append(xt0)

            wsb = []  # wsb[w][it] : [128, OC] bf16
            for w in range(5):
                tiles = []
                for it in range(KT):
                    t = consts.tile([128, OC], bf16, tag=f"w{w}_{it}")
                    nc.sync.dma_start(out=t[:], in_=wT[w][it * 128:(it + 1) * 128, :])
                    tiles.append(t)
                wsb.append(tiles)

            sb_bq = consts.tile([128, OT], f32, tag="bq")
            nc.sync.dma_start(out=sb_bq[:], in_=bq_pp[:])
            sb_bk = consts.tile([128, OT], f32, tag="bk")
            nc.sync.dma_start(out=sb_bk[:], in_=bk_pp[:])
            sb_bv = consts.tile([128, OC], f32, tag="bv")
            nc.sync.dma_start(out=sb_bv[:], in_=bv_bc[:])
            sb_bb = consts.tile([128, OC], f32, tag="bb")
            nc.sync.dma_start(out=sb_bb[:], in_=bb_bc[:])
            sb_ba = consts.tile([128, OC], f32, tag="ba")
            nc.sync.dma_start(out=sb_ba[:], in_=ba_bc[:])
            sb_mask = consts.tile([128, 128], f32, tag="mask")
            nc.sync.dma_start(out=sb_mask[:], in_=mask_ut[:])
            sb_tri = consts.tile([128, 128], f32, tag="tri")
            nc.sync.dma_start(out=sb_tri[:], in_=tri_sc[:])
            sb_ones = consts.tile([128, 128], f32, tag="ones")
            nc.sync.dma_start(out=sb_ones[:], in_=ones_sc[:])

            # per-pair state: [128, 64] (heads 2g, 2g+1 stacked on partitions)
            st32 = []
            st16 = []
            for g in range(OT):
                s32 = consts.tile([128, HEAD_DIM], f32, tag=f"st32_{g}")
                s16 = consts.tile([128, HEAD_DIM], bf16, tag=f"st16_{g}")
                nc.vector.memset(s32[:], 0.0)
                nc.vector.memset(s16[:], 0.0)
                st32.append(s32)
                st16.append(s16)

            # ---- main loop over stripes ------------------------------------
            for s in range(N_STRIPES):
                t0 = s * STRIPE

                # load xT stripe (stripe 0 was preloaded before the weights)
                if s == 0:
                    xsb = xsb0
                else:
                    xsb = []
                    for it in range(KT):
                        xt = xpool.tile([128, STRIPE], bf16, tag=f"x{it}")
                        nc.sync.dma_start(out=xt[:], in_=xT[it * 128:(it + 1) * 128, t0:t0 + STRIPE])
                        xsb.append(xt)

                # transposed projections -> qT, kT  ([o-part, t] bf16)
                qT, kT_ = [], []
                for w in range(2):
                    bias = sb_bq if w == 0 else sb_bk
                    for ot in range(OT):
                        ps = ps_proj.tile([128, STRIPE], f32, tag="proj")
                        for it in range(KT):
                            nc.tensor.matmul(
                                ps[:],
                                lhsT=wsb[w][it][:, ot * 128:(ot + 1) * 128],
                                rhs=xsb[it][:],
                                start=(it == 0),
                                stop=(it == KT - 1),
                            )
                        dst = qkpool.tile([128, STRIPE], bf16, tag=f"qk{w}_{ot}")
                        nc.scalar.activation(
                            dst[:], ps[:], ACT.Identity,
                            bias=bias[:, ot:ot + 1], scale=QSCALE,
                        )
                        (qT if w == 0 else kT_).append(dst)

                # k natural layout via bf16 DMA transpose of kT
                knat = []
                for tt in range(NT):
                    kn = actpool.tile([128, OC], bf16, tag=f"knat{tt}")
                    for ot in range(OT):
                        nc.sync.dma_start(
                            out=kn[:, ot * 128:(ot + 1) * 128],
                            in_=kT_[ot][:, tt * 128:(tt + 1) * 128],
                            transpose=True,
                        )
                    knat.append(kn)

                # natural projections (v, beta, alpha) + per-chunk stats
                vts, vbs, vbms, lasums = [], [], [], []
                for tt in range(NT):
                    psn = {}
                    for w in (2, 3, 4):
                        ps = ps_proj.tile([128, OC], f32, tag="proj")
                        for it in range(KT):
                            nc.tensor.matmul(
                                ps[:],
                                lhsT=xsb[it][:, tt * 128:(tt + 1) * 128],
                                rhs=wsb[w][it][:],
                                start=(it == 0),
                                stop=(it == KT - 1),
                            )
                        psn[w] = ps

                    # v (keep f32 for precise vb/vbm products)
                    v_t = scratch.tile([128, OC], f32, tag="v")
                    nc.vector.tensor_tensor(v_t[:], psn[2][:], sb_bv[:], ALU.add)

                    # beta = softplus(pre + bias) = ln(1 + exp(pre + bias))
                    # (only Exp/Ln/Identity ACT funcs are used anywhere in the
                    # kernel: they share one activation table -> no reloads)
                    tb = scratch.tile([128, OC], f32, tag="tb")
                    nc.vector.tensor_tensor(tb[:], psn[3][:], sb_bb[:], ALU.add)
                    ebt = scratch.tile([128, OC], f32, tag="ebt")
                    nc.scalar.activation(ebt[:], tb[:], ACT.Exp)
                    beta_t = scratch.tile([128, OC], f32, tag="beta")
                    nc.scalar.activation(beta_t[:], ebt[:], ACT.Ln, bias=1.0)

                    # alpha: clip(pre + bias, +-10)
                    ta = scratch.tile([128, OC], f32, tag="ta")
                    nc.vector.tensor_tensor(ta[:], psn[4][:], sb_ba[:], ALU.add)
                    tcl = scratch.tile([128, OC], f32, tag="tcl")
                    nc.vector.tensor_scalar(tcl[:], ta[:], 10.0, -10.0, ALU.min, ALU.max)

                    # u = exp(-logits); -log_alpha = ln(1+u);
                    # alpha_val = sigmoid(logits) = exp(log_alpha) = exp(-ln(1+u))
                    ua = scratch.tile([128, OC], f32, tag="ua")
                    nc.scalar.activation(ua[:], tcl[:], ACT.Exp, scale=-1.0)
                    sp_t = scratch.tile([128, OC], f32, tag="sp")
                    nc.scalar.activation(sp_t[:], ua[:], ACT.Ln, bias=1.0)
                    av_t = outpool.tile([128, OC], f32, tag="av")
                    nc.scalar.activation(av_t[:], sp_t[:], ACT.Exp, scale=-1.0)
                    nc.sync.dma_start(
                        out=av_out[t0 + tt * 128: t0 + (tt + 1) * 128, :], in_=av_t[:]
                    )
                    lasum = small.tile([128, H // HG], f32, tag=f"lasum{tt}")
                    nc.vector.tensor_reduce(
                        lasum[:],
                        sp_t[:].rearrange("p (h d) -> p h d", h=H // HG),
                        AX.X, ALU.add,
                    )

                    # bmean = sum_d beta / 64
                    bsum = small.tile([128, H // HG], f32, tag=f"bsum{tt}")
                    nc.vector.tensor_reduce(
                        bsum[:],
                        beta_t[:].rearrange("p (h d) -> p h d", h=H // HG),
                        AX.X, ALU.add,
                    )
                    bm = small.tile([128, H // HG], f32, tag=f"bm{tt}")
                    nc.vector.tensor_scalar(bm[:], bsum[:], 1.0 / HEAD_DIM, None, ALU.mult)

                    # vb = v*beta (bf16), vbm = v*bmean (bf16)
                    vb_t = actpool.tile([128, OC], bf16, tag=f"vb{tt}")
                    nc.vector.tensor_tensor(vb_t[:], v_t[:], beta_t[:], ALU.mult)
                    vbm_t = actpool.tile([128, OC], bf16, tag=f"vbm{tt}")
                    nc.vector.tensor_tensor(
                        vbm_t[:].rearrange("p (h d) -> p h d", h=H // HG),
                        v_t[:].rearrange("p (h d) -> p h d", h=H // HG),
                        bm[:, :, None].to_broadcast((128, H // HG, HEAD_DIM)),
                        ALU.mult,
                    )
                    vts.append(v_t)
                    vbs.append(vb_t)
                    vbms.append(vbm_t)
                    lasums.append(lasum)

                # ---- attention per chunk (chunk == t-tile) ------------------
                for cc in range(CPS):
                    tsl = cc * 128
                    lasum = lasums[cc]

                    # m[c,h] (cumulative mean log-alpha) and its all-rows bcast
                    ps_m = ps_small.tile([128, H // HG], f32, tag="sm")
                    nc.tensor.matmul(ps_m[:], lhsT=sb_tri[:], rhs=lasum[:])
                    ps_ms = ps_small.tile([128, H // HG], f32, tag="sm")
                    nc.tensor.matmul(ps_ms[:], lhsT=sb_ones[:], rhs=lasum[:])

                    curve = small.tile([128, H // HG], f32, tag="curve")
                    nc.scalar.activation(curve[:], ps_m[:], ACT.Exp)
                    invc = small.tile([128, H // HG], f32, tag="invc")
                    nc.scalar.activation(invc[:], ps_m[:], ACT.Exp, scale=-1.0)
                    asum = small.tile([128, H // HG], f32, tag="asum")
                    nc.scalar.activation(asum[:], ps_ms[:], ACT.Exp)

                    y_f = outpool.tile([128, OC], f32, tag="yf")

                    for g in range(OT):
                        qg, kg = qT[g], kT_[g]
                        # attn_T[j,i] for both heads of the pair
                        ps_a = [None, None]
                        for hh in range(2):
                            r0, r1 = hh * 64, (hh + 1) * 64
                            pa = ps_attn.tile([128, 128], f32, tag="attn")
                            nc.tensor.matmul(
                                pa[:],
                                lhsT=kg[r0:r1, tsl:tsl + 128],
                                rhs=qg[r0:r1, tsl:tsl + 128],
                            )
                            ps_a[hh] = pa

                        for hh in range(2):
                            h = 2 * g + hh
                            r0, r1 = hh * 64, (hh + 1) * 64
                            # scale cols... rows j by invcurve_j, mask j<=i, ->bf16
                            at0 = scratch.tile([128, 128], f32, tag="at0")
                            nc.scalar.activation(
                                at0[:], ps_a[hh][:], ACT.Identity,
                                scale=invc[:, h:h + 1],
                            )
                            atm = scratch.tile([128, 128], bf16, tag="atm")
                            nc.vector.tensor_tensor(atm[:], at0[:], sb_mask[:], ALU.mult)

                            # y_psum = attn_tilde^T . vb  +  qT^T . state
                            ps_y = ps_small.tile([128, HEAD_DIM], f32, tag="sm")
                            nc.tensor.matmul(
                                ps_y[:],
                                lhsT=atm[:],
                                rhs=vbs[cc][:, h * 64:(h + 1) * 64],
                                start=True, stop=False,
                            )
                            nc.tensor.matmul(
                                ps_y[:],
                                lhsT=qg[r0:r1, tsl:tsl + 128],
                                rhs=st16[g][r0:r1, :],
                                start=False, stop=True,
                            )
                            # y = curve_i * psum
                            nc.scalar.activation(
                                y_f[:, h * 64:(h + 1) * 64], ps_y[:], ACT.Identity,
                                scale=curve[:, h:h + 1],
                            )

                        # delta for the pair (col-tiled outer products)
                        ps_d = ps_small.tile([128, HEAD_DIM], f32, tag="sm")
                        for hh in range(2):
                            h = 2 * g + hh
                            nc.tensor.matmul(
                                ps_d[hh * 64:(hh + 1) * 64, :],
                                lhsT=knat[cc][:, h * 64:(h + 1) * 64],
                                rhs=vbms[cc][:, h * 64:(h + 1) * 64],
                            )

                        # state = state * asum + delta   (f32, then bf16 copy)
                        for hh in range(2):
                            h = 2 * g + hh
                            r0, r1 = hh * 64, (hh + 1) * 64
                            nc.vector.tensor_scalar(
                                st32[g][r0:r1, :], st32[g][r0:r1, :],
                                asum[r0:r1, h:h + 1], None, ALU.mult,
                            )
                            nc.vector.tensor_tensor(
                                st32[g][r0:r1, :], st32[g][r0:r1, :],
                                ps_d[r0:r1, :], ALU.add,
                            )
                        nc.vector.tensor_copy(st16[g][:], st32[g][:])

                    nc.sync.dma_start(
                        out=y_out[t0 + tsl: t0 + tsl + 128, :], in_=y_f[:]
                    )

            # final state -> DRAM
            for g in range(OT):
                nc.sync.dma_start(out=st_out[g * 128:(g + 1) * 128, :], in_=st32[g][:])

    nc.compile()
    return nc


def _get_program():
    if "nc" not in _PROGRAM_CACHE:
        _PROGRAM_CACHE["nc"] = build_program()
    return _PROGRAM_CACHE["nc"]


def make_core_inputs(x, Wq, bq, Wk, bk, Wv, bv, Wbeta, bbeta, Walpha, balpha):
    """Host-side shard prep: returns in_maps (list of 8 dicts)."""
    import ml_dtypes

    bf16 = ml_dtypes.bfloat16
    f32 = np.float32

    # constants shared by all cores
    idx = np.arange(128)
    mask_ut = (idx[:, None] <= idx[None, :]).astype(f32)  # [j, i] : j <= i
    # lasum holds -sum_d(log_alpha); fold the sign and the /64 into the
    # cumulative-sum matmul constants
    tri = np.triu(np.ones((128, 128), f32)) * (-1.0 / HEAD_DIM)  # c' <= c
    ones_sc = np.full((128, 128), -1.0 / HEAD_DIM, f32)

    Ws = [Wq, Wk, Wv, Wbeta, Walpha]
    in_maps = []
    for core in range(N_CORES):
        b, hg = divmod(core, HG)
        sl = slice(hg * OC, (hg + 1) * OC)
        m = {
            "xT": np.ascontiguousarray(np.asarray(x[b]).T).astype(bf16),
            "mask_ut": mask_ut,
            "tri_sc": tri,
            "ones_sc": ones_sc,
        }
        for i, W in enumerate(Ws):
            m[f"wT{i}"] = np.ascontiguousarray(np.asarray(W)[sl].T).astype(bf16)
        m["bq_pp"] = np.ascontiguousarray(
            (np.asarray(bq)[sl] * QSCALE).astype(f32).reshape(OT, 128).T
        )
        m["bk_pp"] = np.ascontiguousarray(
            (np.asarray(bk)[sl] * QSCALE).astype(f32).reshape(OT, 128).T
        )
        m["bv_bc"] = np.ascontiguousarray(
            np.broadcast_to(np.asarray(bv)[sl].astype(f32), (128, OC))
        )
        m["bb_bc"] = np.ascontiguousarray(
            np.broadcast_to(np.asarray(bbeta)[sl].astype(f32), (128, OC))
        )
        m["ba_bc"] = np.ascontiguousarray(
            np.broadcast_to(np.asarray(balpha)[sl].astype(f32), (128, OC))
        )
        in_maps.append(m)
    return in_maps


def assemble_outputs(results):
    """results: list of 8 dicts with y_out/av_out/st_out -> full (y, state, alpha_val)."""
    y = np.zeros((B, T, D), np.float32)
    state = np.zeros((B, H, HEAD_DIM, HEAD_DIM), np.float32)
    alpha_val = np.zeros((B, T, H, HEAD_DIM), np.float32)
    for core in range(N_CORES):
        b, hg = divmod(core, HG)
        r = results[core]
        y[b, :, hg * OC:(hg + 1) * OC] = r["y_out"]
        alpha_val[b, :, hg * 8:(hg + 1) * 8, :] = r["av_out"].reshape(T, 8, HEAD_DIM)
        state[b, hg * 8:(hg + 1) * 8] = r["st_out"].reshape(8, HEAD_DIM, HEAD_DIM)
    return y, state, alpha_val


def _ensure_ntff_hook():
    """This image lacks the antenv.axon_hooks glue module; synthesize it so
    run_bass_kernel_spmd(trace=True) can reach the ctypes NTFF hook."""
    import sys
    import types

    if "antenv.axon_hooks" in sys.modules:
        return
    try:
        import antenv

        mod = types.ModuleType("antenv.axon_hooks")
        _h = [None]
        mod.set_axon_ntff_profile_hook = lambda h: _h.__setitem__(0, h)
        mod.get_axon_ntff_profile_hook = lambda: _h[0]
        sys.modules["antenv.axon_hooks"] = mod
        antenv.axon_hooks = mod
        from trn_agent_boot.trn_boot import _ntff_profile_via_ctypes

        mod.set_axon_ntff_profile_hook(
            _ntff_profile_via_ctypes("/opt/axon/libaxon_pjrt.so")
        )
    except Exception:
        pass


def run(inputs, trace=False):
    global LAST_RESULTS
    import jax  # ensures the axon PJRT platform is initialized

    jax.devices()
    _ensure_ntff_hook()
    from concourse.bass_utils import run_bass_kernel_spmd

    nc = _get_program()
    in_maps = make_core_inputs(**inputs)
    res = run_bass_kernel_spmd(
        nc, in_maps, list(range(N_CORES)), trace=trace
    )
    LAST_RESULTS = res
    return res


def kernel(x, Wq, bq, Wk, bk, Wv, bv, Wbeta, bbeta, Walpha, balpha):
    res = run(dict(
        x=x, Wq=Wq, bq=bq, Wk=Wk, bk=bk, Wv=Wv, bv=bv, Wbeta=Wbeta,
        bbeta=bbeta, Walpha=Walpha, balpha=balpha,
    ))
    return assemble_outputs(res.results)


# revision 16
# speedup vs baseline: 1.3118x; 1.0091x over previous
# Chunkwise delta-rule attention kernel for Trainium2 (Bass/Tile), 8 NeuronCores.
#
# Sharding: 8 cores = 4 batches x 2 head-groups (8 heads each). Each core:
#   - projects x[b] (4096x1024) against its 512-row weight slices (q,k,v,beta,alpha)
#   - runs the chunked intra-attention + sequential inter-chunk state scan
#   - produces y[b,:,hg*512:(hg+1)*512], state[b, hg*8:(hg+1)*8], alpha_val slice
#
# Layout strategy (contraction dim must be the SBUF partition dim for matmul):
#   - x and W are pre-transposed and cast to bf16 on the host -> xT [1024,4096],
#     wT [1024,512]. q/k are produced transposed ([head_dim, t]) for the attn
#     matmul; v/beta/alpha are produced natural ([t, dim]). k_nat is recovered
#     from kT with on-device bf16 DMA transposes.
#   - decay exp(m_i - m_j) is factored as curve_i * invcurve_j; curve_i is
#     folded into the final per-row y scale, invcurve_j into the attn matrix.
#   - cumsum over chunk positions is a matmul with a triangular constant.

import numpy as np

B, T, D = 4, 4096, 1024
HEAD_DIM, CHUNK = 64, 128
H = D // HEAD_DIM  # 16 heads total
HG = 2  # head groups (cores per batch)
OC = D // HG  # 512 per-core projected dim
N_CORES = 8
STRIPE = 512
N_STRIPES = T // STRIPE  # 8
CPS = STRIPE // CHUNK  # chunks per stripe = 4
NT = STRIPE // 128  # t-tiles per stripe = 4
KT = D // 128  # k(contraction) tiles = 8
OT = OC // 128  # o-tiles = 4 (one per head pair)
QSCALE = float(HEAD_DIM) ** -0.5  # 0.125

_PROGRAM_CACHE = {}
LAST_RESULTS = None  # BassKernelResults of the most recent run (for test harness)


def build_program():
    import concourse.mybir as mybir
    import concourse.tile as tile
    from concourse import bacc

    f32 = mybir.dt.float32
    bf16 = mybir.dt.bfloat16
    ALU = mybir.AluOpType
    ACT = mybir.ActivationFunctionType
    AX = mybir.AxisListType

    # The act-table chooser greedily picks the first act_info.json set that
    # contains each function, which thrashes between the exp-only and ln-only
    # tables. Every ACT func this kernel uses (Exp, Ln, Identity, Copy) lives
    # in 'natural_log_exp_and_others'; blank out the other sets (keeping list
    # positions, since act_func_set_id is the index) so one table is loaded
    # once and never evicted.
    import concourse.bacc as bacc_mod

    if not hasattr(bacc_mod, "_orig_get_activation_tables"):
        bacc_mod._orig_get_activation_tables = bacc_mod.get_activation_tables

        def _only_ln_exp_tables(module_arch):
            tabs = bacc_mod._orig_get_activation_tables(module_arch)
            return {
                name: (s if name == "natural_log_exp_and_others" else set())
                for name, s in tabs.items()
            }

        bacc_mod.get_activation_tables = _only_ln_exp_tables

    nc = bacc.Bacc()

    # ---- DRAM I/O -----------------------------------------------------------
    xT = nc.declare_dram_parameter("xT", [D, T], bf16, isOutput=False)
    wT = [
        nc.declare_dram_parameter(f"wT{i}", [D, OC], bf16, isOutput=False)
        for i in range(5)
    ]  # order: q, k, v, beta, alpha
    bq_pp = nc.declare_dram_parameter("bq_pp", [128, OT], f32, isOutput=False)
    bk_pp = nc.declare_dram_parameter("bk_pp", [128, OT], f32, isOutput=False)
    bv_bc = nc.declare_dram_parameter("bv_bc", [128, OC], f32, isOutput=False)
    bb_bc = nc.declare_dram_parameter("bb_bc", [128, OC], f32, isOutput=False)
    ba_bc = nc.declare_dram_parameter("ba_bc", [128, OC], f32, isOutput=False)
    mask_ut = nc.declare_dram_parameter("mask_ut", [128, 128], f32, isOutput=False)
    tri_sc = nc.declare_dram_parameter("tri_sc", [128, 128], f32, isOutput=False)
    ones_sc = nc.declare_dram_parameter("ones_sc", [128, 128], f32, isOutput=False)

    y_out = nc.declare_dram_parameter("y_out", [T, OC], f32, isOutput=True)
    av_out = nc.declare_dram_parameter("av_out", [T, OC], f32, isOutput=True)
    st_out = nc.declare_dram_parameter("st_out", [4 * 128, HEAD_DIM], f32, isOutput=True)

    with tile.TileContext(nc) as tc:
        import contextlib

        ctx = contextlib.ExitStack()
        with ctx:
            consts = ctx.enter_context(tc.tile_pool(name="consts", bufs=1))
            xpool = ctx.enter_context(tc.tile_pool(name="xpool", bufs=3))
            qkpool = ctx.enter_context(tc.tile_pool(name="qkpool", bufs=2))
            actpool = ctx.enter_context(tc.tile_pool(name="actpool", bufs=2))
            scratch = ctx.enter_context(tc.tile_pool(name="scratch", bufs=3))
            small = ctx.enter_context(tc.tile_pool(name="small", bufs=3))
            outpool = ctx.enter_context(tc.tile_pool(name="outpool", bufs=2))
            ps_proj = ctx.enter_context(tc.tile_pool(name="ps_proj", bufs=3, space="PSUM"))
            ps_attn = ctx.enter_context(tc.tile_pool(name="ps_attn", bufs=2, space="PSUM"))
            ps_small = ctx.enter_context(tc.tile_pool(name="ps_small", bufs=3, space="PSUM"))

            # ---- constants / weights into SBUF ------------------------------
            # first stripe of xT precedes the weights in DMA priority order so
            # the PE can start projecting as early as possible
            xsb0 = []
            for it in range(KT):
                xt0 = xpool.tile([128, STRIPE], bf16, tag=f"x{it}")
                nc.sync.dma_start(out=xt0[:], in_=xT[it * 128:(it + 1) * 128, 0:STRIPE])
                xsb0.append(xt0)

            wsb = []  # wsb[w][it] : [128, OC] bf16
            for w in range(5):
                tiles = []
                for it in range(KT):
                    t = consts.tile([128, OC], bf16, tag=f"w{w}_{it}")
                    nc.sync.dma_start(out=t[:], in_=wT[w][it * 128:(it + 1) * 128, :])
                    tiles.append(t)
                wsb.append(tiles)

            sb_bq = consts.tile([128, OT], f32, tag="bq")
            nc.sync.dma_start(out=sb_bq[:], in_=bq_pp[:])
            sb_bk = consts.tile([128, OT], f32, tag="bk")
            nc.sync.dma_start(out=sb_bk[:], in_=bk_pp[:])
            sb_bv = consts.tile([128, OC], f32, tag="bv")
            nc.sync.dma_start(out=sb_bv[:], in_=bv_bc[:])
            sb_bb = consts.tile([128, OC], f32, tag="bb")
            nc.sync.dma_start(out=sb_bb[:], in_=bb_bc[:])
            sb_ba = consts.tile([128, OC], f32, tag="ba")
            nc.sync.dma_start(out=sb_ba[:], in_=ba_bc[:])
            sb_mask = consts.tile([128, 128], f32, tag="mask")
            nc.sync.dma_start(out=sb_mask[:], in_=mask_ut[:])
            sb_tri = consts.tile([128, 128], f32, tag="tri")
            nc.sync.dma_start(out=sb_tri[:], in_=tri_sc[:])
            sb_ones = consts.tile([128, 128], f32, tag="ones")
            nc.sync.dma_start(out=sb_ones[:], in_=ones_sc[:])

            # per-pair state: [128, 64] (heads 2g, 2g+1 stacked on partitions)
            st32 = []
            st16 = []
            for g in range(OT):
                s32 = consts.tile([128, HEAD_DIM], f32, tag=f"st32_{g}")
                s16 = consts.tile([128, HEAD_DIM], bf16, tag=f"st16_{g}")
                nc.vector.memset(s32[:], 0.0)
                nc.vector.memset(s16[:], 0.0)
                st32.append(s32)
                st16.append(s16)

            # ---- main loop over stripes ------------------------------------
            for s in range(N_STRIPES):
                t0 = s * STRIPE

                # load xT stripe (stripe 0 was preloaded before the weights)
                if s == 0:
                    xsb = xsb0
                else:
                    xsb = []
                    for it in range(KT):
                        xt = xpool.tile([128, STRIPE], bf16, tag=f"x{it}")
                        nc.sync.dma_start(out=xt[:], in_=xT[it * 128:(it + 1) * 128, t0:t0 + STRIPE])
                        xsb.append(xt)

                # transposed projections -> qT, kT  ([o-part, t] bf16)
                qT, kT_ = [], []
                for w in range(2):
                    bias = sb_bq if w == 0 else sb_bk
                    for ot in range(OT):
                        ps = ps_proj.tile([128, STRIPE], f32, tag="proj")
                        for it in range(KT):
                            nc.tensor.matmul(
                                ps[:],
                                lhsT=wsb[w][it][:, ot * 128:(ot + 1) * 128],
                                rhs=xsb[it][:],
                                start=(it == 0),
                                stop=(it == KT - 1),
                            )
                        dst = qkpool.tile([128, STRIPE], bf16, tag=f"qk{w}_{ot}")
                        nc.scalar.activation(
                            dst[:], ps[:], ACT.Identity,
                            bias=bias[:, ot:ot + 1], scale=QSCALE,
                        )
                        (qT if w == 0 else kT_).append(dst)

                # k natural layout via bf16 DMA transpose of kT
                knat = []
                for tt in range(NT):
                    kn = actpool.tile([128, OC], bf16, tag=f"knat{tt}")
                    for ot in range(OT):
                        nc.sync.dma_start(
                            out=kn[:, ot * 128:(ot + 1) * 128],
                            in_=kT_[ot][:, tt * 128:(tt + 1) * 128],
                            transpose=True,
                        )
                    knat.append(kn)

                # natural projections (v, beta, alpha) + per-chunk stats
                vts, vbs, vbms, lasums = [], [], [], []
                for tt in range(NT):
                    psn = {}
                    for w in (2, 3, 4):
                        ps = ps_proj.tile([128, OC], f32, tag="proj")
                        for it in range(KT):
                            nc.tensor.matmul(
                                ps[:],
                                lhsT=xsb[it][:, tt * 128:(tt + 1) * 128],
                                rhs=wsb[w][it][:],
                                start=(it == 0),
                                stop=(it == KT - 1),
                            )
                        psn[w] = ps

                    # v (keep f32 for precise vb/vbm products)
                    v_t = scratch.tile([128, OC], f32, tag="v")
                    nc.vector.tensor_tensor(v_t[:], psn[2][:], sb_bv[:], ALU.add)

                    # beta = softplus(pre + bias) = ln(1 + exp(pre + bias))
                    # (only Exp/Ln/Identity ACT funcs are used anywhere in the
                    # kernel: they share one activation table -> no reloads)
                    tb = scratch.tile([128, OC], f32, tag="tb")
                    nc.vector.tensor_tensor(tb[:], psn[3][:], sb_bb[:], ALU.add)
                    ebt = scratch.tile([128, OC], f32, tag="ebt")
                    nc.scalar.activation(ebt[:], tb[:], ACT.Exp)
                    beta_t = scratch.tile([128, OC], f32, tag="beta")
                    nc.scalar.activation(beta_t[:], ebt[:], ACT.Ln, bias=1.0)

                    # alpha: clip(pre + bias, +-10)
                    ta = scratch.tile([128, OC], f32, tag="ta")
                    nc.vector.tensor_tensor(ta[:], psn[4][:], sb_ba[:], ALU.add)
                    tcl = scratch.tile([128, OC], f32, tag="tcl")
                    nc.vector.tensor_scalar(tcl[:], ta[:], 10.0, -10.0, ALU.min, ALU.max)

                    # u = exp(-logits); -log_alpha = ln(1+u);
                    # alpha_val = sigmoid(logits) = exp(log_alpha) = exp(-ln(1+u))
                    ua = scratch.tile([128, OC], f32, tag="ua")
                    nc.scalar.activation(ua[:], tcl[:], ACT.Exp, scale=-1.0)
                    sp_t = scratch.tile([128, OC], f32, tag="sp")
                    nc.scalar.activation(sp_t[:], ua[:], ACT.Ln, bias=1.0)
                    av_t = outpool.tile([128, OC], f32, tag="av")
                    nc.scalar.activation(av_t[:], sp_t[:], ACT.Exp, scale=-1.0)
                    nc.sync.dma_start(
                        out=av_out[t0 + tt * 128: t0 + (tt + 1) * 128, :], in_=av_t[:]
                    )
                    lasum = small.tile([128, H // HG], f32, tag=f"lasum{tt}")
                    nc.vector.tensor_reduce(
                        lasum[:],
                        sp_t[:].rearrange("p (h d) -> p h d", h=H // HG),
                        AX.X, ALU.add,
                    )

                    # bmean = sum_d beta / 64
                    bsum = small.tile([128, H // HG], f32, tag=f"bsum{tt}")
                    nc.vector.tensor_reduce(
                        bsum[:],
                        beta_t[:].rearrange("p (h d) -> p h d", h=H // HG),
                        AX.X, ALU.add,
                    )
                    bm = small.tile([128, H // HG], f32, tag=f"bm{tt}")
                    nc.vector.tensor_scalar(bm[:], bsum[:], 1.0 / HEAD_DIM, None, ALU.mult)

                    # vb = v*beta (bf16), vbm = v*bmean (bf16)
                    vb_t = actpool.tile([128, OC], bf16, tag=f"vb{tt}")
                    nc.vector.tensor_tensor(vb_t[:], v_t[:], beta_t[:], ALU.mult)
                    vbm_t = actpool.tile([128, OC], bf16, tag=f"vbm{tt}")
                    nc.vector.tensor_tensor(
                        vbm_t[:].rearrange("p (h d) -> p h d", h=H // HG),
                        v_t[:].rearrange("p (h d) -> p h d", h=H // HG),
                        bm[:, :, None].to_broadcast((128, H // HG, HEAD_DIM)),
                        ALU.mult,
                    )
                    vts.append(v_t)
                    vbs.append(vb_t)
                    vbms.append(vbm_t)
                    lasums.append(lasum)

                # ---- attention per chunk (chunk == t-tile) ------------------
                for cc in range(CPS):
                    tsl = cc * 128
                    lasum = lasums[cc]

                    # m[c,h] (cumulative mean log-alpha) and its all-rows bcast
                    ps_m = ps_small.tile([128, H // HG], f32, tag="sm")
                    nc.tensor.matmul(ps_m[:], lhsT=sb_tri[:], rhs=lasum[:])
                    ps_ms = ps_small.tile([128, H // HG], f32, tag="sm")
                    nc.tensor.matmul(ps_ms[:], lhsT=sb_ones[:], rhs=lasum[:])

                    curve = small.tile([128, H // HG], f32, tag="curve")
                    nc.scalar.activation(curve[:], ps_m[:], ACT.Exp)
                    invc = small.tile([128, H // HG], f32, tag="invc")
                    nc.scalar.activation(invc[:], ps_m[:], ACT.Exp, scale=-1.0)
                    asum = small.tile([128, H // HG], f32, tag="asum")
                    nc.scalar.activation(asum[:], ps_ms[:], ACT.Exp)

                    y_f = outpool.tile([128, OC], f32, tag="yf")

                    for g in range(OT):
                        qg, kg = qT[g], kT_[g]
                        # attn_T[j,i] for both heads of the pair
                        ps_a = [None, None]
                        for hh in range(2):
                            r0, r1 = hh * 64, (hh + 1) * 64
                            pa = ps_attn.tile([128, 128], f32, tag="attn")
                            nc.tensor.matmul(
                                pa[:],
                                lhsT=kg[r0:r1, tsl:tsl + 128],
                                rhs=qg[r0:r1, tsl:tsl + 128],
                            )
                            ps_a[hh] = pa

                        for hh in range(2):
                            h = 2 * g + hh
                            r0, r1 = hh * 64, (hh + 1) * 64
                            # scale cols... rows j by invcurve_j, mask j<=i, ->bf16
                            at0 = scratch.tile([128, 128], f32, tag="at0")
                            nc.scalar.activation(
                                at0[:], ps_a[hh][:], ACT.Identity,
                                scale=invc[:, h:h + 1],
                            )
                            atm = scratch.tile([128, 128], bf16, tag="atm")
                            nc.vector.tensor_tensor(atm[:], at0[:], sb_mask[:], ALU.mult)

                            # y_psum = attn_tilde^T . vb  +  qT^T . state
                            ps_y = ps_small.tile([128, HEAD_DIM], f32, tag="sm")
                            nc.tensor.matmul(
                                ps_y[:],
                                lhsT=atm[:],
                                rhs=vbs[cc][:, h * 64:(h + 1) * 64],
                                start=True, stop=False,
                            )
                            nc.tensor.matmul(
                                ps_y[:],
                                lhsT=qg[r0:r1, tsl:tsl + 128],
                                rhs=st16[g][r0:r1, :],
                                start=False, stop=True,
                            )
                            # y = curve_i * psum  (DVE: keeps ACT off this path)
                            nc.vector.tensor_scalar(
                                y_f[:, h * 64:(h + 1) * 64], ps_y[:],
                                curve[:, h:h + 1], None, ALU.mult,
                            )

                        # delta for the pair (col-tiled outer products)
                        ps_d = ps_small.tile([128, HEAD_DIM], f32, tag="sm")
                        for hh in range(2):
                            h = 2 * g + hh
                            nc.tensor.matmul(
                                ps_d[hh * 64:(hh + 1) * 64, :],
                                lhsT=knat[cc][:, h * 64:(h + 1) * 64],
                                rhs=vbms[cc][:, h * 64:(h + 1) * 64],
                            )

                        # state = state * asum + delta   (f32, then bf16 copy)
                        for hh in range(2):
                            h = 2 * g + hh
                            r0, r1 = hh * 64, (hh + 1) * 64
                            nc.vector.tensor_scalar(
                                st32[g][r0:r1, :], st32[g][r0:r1, :],
                                asum[r0:r1, h:h + 1], None, ALU.mult,
                            )
                            nc.vector.tensor_tensor(
                                st32[g][r0:r1, :], st32[g][r0:r1, :],
                                ps_d[r0:r1, :], ALU.add,
                            )
                        nc.vector.tensor_copy(st16[g][:], st32[g][:])

                    nc.sync.dma_start(
                        out=y_out[t0 + tsl: t0 + tsl + 128, :], in_=y_f[:]
                    )

            # final state -> DRAM
            for g in range(OT):
                nc.sync.dma_start(out=st_out[g * 128:(g + 1) * 128, :], in_=st32[g][:])

    nc.compile()
    return nc


def _get_program():
    if "nc" not in _PROGRAM_CACHE:
        _PROGRAM_CACHE["nc"] = build_program()
    return _PROGRAM_CACHE["nc"]


def make_core_inputs(x, Wq, bq, Wk, bk, Wv, bv, Wbeta, bbeta, Walpha, balpha):
    """Host-side shard prep: returns in_maps (list of 8 dicts)."""
    import ml_dtypes

    bf16 = ml_dtypes.bfloat16
    f32 = np.float32

    # constants shared by all cores
    idx = np.arange(128)
    mask_ut = (idx[:, None] <= idx[None, :]).astype(f32)  # [j, i] : j <= i
    # lasum holds -sum_d(log_alpha); fold the sign and the /64 into the
    # cumulative-sum matmul constants
    tri = np.triu(np.ones((128, 128), f32)) * (-1.0 / HEAD_DIM)  # c' <= c
    ones_sc = np.full((128, 128), -1.0 / HEAD_DIM, f32)

    Ws = [Wq, Wk, Wv, Wbeta, Walpha]
    in_maps = []
    for core in range(N_CORES):
        b, hg = divmod(core, HG)
        sl = slice(hg * OC, (hg + 1) * OC)
        m = {
            "xT": np.ascontiguousarray(np.asarray(x[b]).T).astype(bf16),
            "mask_ut": mask_ut,
            "tri_sc": tri,
            "ones_sc": ones_sc,
        }
        for i, W in enumerate(Ws):
            m[f"wT{i}"] = np.ascontiguousarray(np.asarray(W)[sl].T).astype(bf16)
        m["bq_pp"] = np.ascontiguousarray(
            (np.asarray(bq)[sl] * QSCALE).astype(f32).reshape(OT, 128).T
        )
        m["bk_pp"] = np.ascontiguousarray(
            (np.asarray(bk)[sl] * QSCALE).astype(f32).reshape(OT, 128).T
        )
        m["bv_bc"] = np.ascontiguousarray(
            np.broadcast_to(np.asarray(bv)[sl].astype(f32), (128, OC))
        )
        m["bb_bc"] = np.ascontiguousarray(
            np.broadcast_to(np.asarray(bbeta)[sl].astype(f32), (128, OC))
        )
        m["ba_bc"] = np.ascontiguousarray(
            np.broadcast_to(np.asarray(balpha)[sl].astype(f32), (128, OC))
        )
        in_maps.append(m)
    return in_maps


def assemble_outputs(results):
    """results: list of 8 dicts with y_out/av_out/st_out -> full (y, state, alpha_val)."""
    y = np.zeros((B, T, D), np.float32)
    state = np.zeros((B, H, HEAD_DIM, HEAD_DIM), np.float32)
    alpha_val = np.zeros((B, T, H, HEAD_DIM), np.float32)
    for core in range(N_CORES):
        b, hg = divmod(core, HG)
        r = results[core]
        y[b, :, hg * OC:(hg + 1) * OC] = r["y_out"]
        alpha_val[b, :, hg * 8:(hg + 1) * 8, :] = r["av_out"].reshape(T, 8, HEAD_DIM)
        state[b, hg * 8:(hg + 1) * 8] = r["st_out"].reshape(8, HEAD_DIM, HEAD_DIM)
    return y, state, alpha_val


def _ensure_ntff_hook():
    """This image lacks the antenv.axon_hooks glue module; synthesize it so
    run_bass_kernel_spmd(trace=True) can reach the ctypes NTFF hook."""
    import sys
    import types

    if "antenv.axon_hooks" in sys.modules:
        return
    try:
        import antenv

        mod = types.ModuleType("antenv.axon_hooks")
        _h = [None]
        mod.set_axon_ntff_profile_hook = lambda h: _h.__setitem__(0, h)
        mod.get_axon_ntff_profile_hook = lambda: _h[0]
        sys.modules["antenv.axon_hooks"] = mod
        antenv.axon_hooks = mod
        from trn_agent_boot.trn_boot import _ntff_profile_via_ctypes

        mod.set_axon_ntff_profile_hook(
            _ntff_profile_via_ctypes("/opt/axon/libaxon_pjrt.so")
        )
    except Exception:
        pass


def run(inputs, trace=False):
    global LAST_RESULTS
    import jax  # ensures the axon PJRT platform is initialized

    jax.devices()
    _ensure_ntff_hook()
    from concourse.bass_utils import run_bass_kernel_spmd

    nc = _get_program()
    in_maps = make_core_inputs(**inputs)
    res = run_bass_kernel_spmd(
        nc, in_maps, list(range(N_CORES)), trace=trace
    )
    LAST_RESULTS = res
    return res


def kernel(x, Wq, bq, Wk, bk, Wv, bv, Wbeta, bbeta, Walpha, balpha):
    res = run(dict(
        x=x, Wq=Wq, bq=bq, Wk=Wk, bk=bk, Wv=Wv, bv=bv, Wbeta=Wbeta,
        bbeta=bbeta, Walpha=Walpha, balpha=balpha,
    ))
    return assemble_outputs(res.results)
